# revision 1
# baseline (speedup 1.0000x reference)
"""Trainium2 Bass kernel for nn_EquivariantNodeFFN (equivariant gated FFN).

Strategy (pure data parallel over nodes, 8 cores x 8192 nodes):
  - Per core, process 16 blocks of 512 nodes (4 subtiles of 128).
  - Node-major x is loaded once; norm stats computed on-chip (DVE bn_stats +
    fused square-reduce); the per-node normalization scales are fused into
    the node-major fp32->bf16 casts (tensor_scalar with per-partition scale),
    then PE transposes (with tile_position col/row packing) produce the
    feature-major bf16 activations.
  - lin1/lin2 run feature-major with the tiny per-irrep weights as the
    stationary matmul operand (bf16).  Gates use tanh (sigmoid(x) =
    0.5*(1+tanh(x/2)); the 0.5 is folded into V1/V2), so a single ACT
    table set ("silu_and_others": silu + tanh + copy + identity) is used.
  - rsqrt for the norm is computed on DVE with a Quake-seed + Newton
    iterations (ACT Sqrt lives in a different table set; switching costs
    2.7us per block).
  - Device output is t*dx in feature-major (m-major row order); the host
    adds the residual x and un-permutes columns (gather/unshard step).
"""

import os
import sys

sys.path.insert(0, "/opt/trn_rl_repo")

import numpy as np
import ml_dtypes

import concourse.bass as bass
import concourse.bacc as bacc
import concourse.tile as tile
from concourse import mybir
from concourse.bass_utils import run_bass_kernel_spmd

F32 = mybir.dt.float32
BF16 = mybir.dt.bfloat16
I32 = mybir.dt.int32
AF = mybir.ActivationFunctionType
OP = mybir.AluOpType

# ---- problem constants (hardcoded per contract) ----
N_NODES = 65536
N_CORES = 8
NC = N_NODES // N_CORES      # 8192 nodes per core
BLK = 512                    # nodes per block
NSUB = 4                     # subtiles per block
SUB = 128                    # nodes per subtile
NBLK = NC // BLK             # 16

M0, M1, M2 = 128, 64, 32
H0, H1, H2 = 512, 256, 128
G = H1 + H2                  # 384
D_IN = M0 + 3 * M1 + 5 * M2  # 480
EPS = 1e-8
S0, S1, S2 = float(np.sqrt(M0)), float(np.sqrt(M1)), float(np.sqrt(M2))
T0, T1, T2 = float(np.sqrt(H0)), float(np.sqrt(H1)), float(np.sqrt(H2))

MAGIC = 0x5F3759DF

# device feature-row order (m-major within each degree) -> x column
PERM = np.array(
    list(range(M0))
    + [M0 + 3 * u + m for m in range(3) for u in range(M1)]
    + [M0 + 3 * M1 + 5 * u + m for m in range(5) for u in range(M2)]
)

_BUILT = None
TRACE = False
TRACE_KW = {}
LAST_RESULTS = None
# debug probes (timing bisects; break correctness when set)
P_SKIP_GATING = os.environ.get("P_SKIP_GATING") == "1"
P_SKIP_TINY = os.environ.get("P_SKIP_TINY") == "1"
P_SKIP_PE = os.environ.get("P_SKIP_PE") == "1"
P_SKIP_STATS = os.environ.get("P_SKIP_STATS") == "1"
P_SKIP_ACTF = os.environ.get("P_SKIP_ACTF") == "1"


def _build_bass(nrep=1):
    nc = bacc.Bacc("TRN2", target_bir_lowering=False)

    x_d = nc.dram_tensor("x", [NC, D_IN], F32, kind="ExternalInput")
    w0_d = nc.dram_tensor("w0", [128, 7, 128], BF16, kind="ExternalInput")
    w1_d = nc.dram_tensor("w1", [128, 2, 128], BF16, kind="ExternalInput")  # replicated x2 on partitions
    w2_d = nc.dram_tensor("w2", [128, 128], BF16, kind="ExternalInput")     # replicated x4 on partitions
    v0_d = nc.dram_tensor("v0", [128, 4, 128], BF16, kind="ExternalInput")
    v1_d = nc.dram_tensor("v1", [128, 2, 64], BF16, kind="ExternalInput")
    v2_d = nc.dram_tensor("v2", [128, 32], BF16, kind="ExternalInput")
    b0_d = nc.dram_tensor("b0", [128, 7], F32, kind="ExternalInput")
    c0_d = nc.dram_tensor("c0", [128, 1], F32, kind="ExternalInput")
    eye_d = nc.dram_tensor("eye", [128, 128], BF16, kind="ExternalInput")
    o_d = nc.dram_tensor("o", [D_IN, NC], F32, kind="ExternalOutput")

    with tile.TileContext(nc) as tc:
        with (
            tc.tile_pool(name="const", bufs=1) as const,
            tc.tile_pool(name="xin", bufs=12) as xin,
            tc.tile_pool(name="xb", bufs=10) as xbp,
            tc.tile_pool(name="stat", bufs=10) as statp,
            tc.tile_pool(name="blkstat", bufs=3) as bstat,
            tc.tile_pool(name="ysb", bufs=6) as ysb,
            tc.tile_pool(name="act", bufs=8) as actp,
            tc.tile_pool(name="z", bufs=28) as zp,
            tc.tile_pool(name="ofm", bufs=3) as ofmp,
            tc.tile_pool(name="dump", bufs=1) as dumpp,
            tc.tile_pool(name="tp", bufs=3, space="PSUM") as tpp,
            tc.tile_pool(name="hp", bufs=5, space="PSUM") as hpp,
        ):
            # constants
            w0s = const.tile([128, 7, 128], BF16)
            w1s = const.tile([128, 2, 128], BF16)
            w2s = const.tile([128, 128], BF16)
            v0s = const.tile([128, 4, 128], BF16)
            v1s = const.tile([128, 2, 64], BF16)
            v2s = const.tile([128, 32], BF16)
            b0s = const.tile([128, 7], F32)
            c0s = const.tile([128, 1], F32)
            eye = const.tile([128, 128], BF16)
            magic = const.tile([128, 8], I32)
            cneg = const.tile([128, 8], F32)   # -0.5
            c15 = const.tile([128, 8], F32)    # 1.5
            keps = const.tile([128, 8], F32)   # EPS
            k384 = const.tile([128, 4], F32)   # 1/384
            k320 = const.tile([128, 4], F32)   # 1/320
            k128 = const.tile([128, 4], F32)   # 1/128
            for sb, dr in ((w0s, w0_d), (w1s, w1_d), (w2s, w2_d), (v0s, v0_d),
                           (v1s, v1_d), (v2s, v2_d), (b0s, b0_d), (c0s, c0_d),
                           (eye, eye_d)):
                nc.sync.dma_start(out=sb[:], in_=dr[:])
            nc.vector.memset(magic[:], MAGIC)
            nc.gpsimd.memset(cneg[:], -0.5)
            nc.gpsimd.memset(c15[:], 1.5)
            nc.gpsimd.memset(keps[:], EPS)
            nc.gpsimd.memset(k384[:], 1.0 / 384.0)
            nc.gpsimd.memset(k320[:], 1.0 / 320.0)
            nc.gpsimd.memset(k128[:], 1.0 / 128.0)

            dump = dumpp.tile([128, 352], F32)
            dumpb = dumpp.tile([128, 352], BF16)

            for b in range(NBLK * nrep):
                j0 = (b % NBLK) * BLK
                x_s = []
                xb_s = []
                xc_s = []
                mv_s = []
                # block-level stats: cols 0:4 var0(s)+eps, 4:8 q(s)
                vq = bstat.tile([128, 8], F32, tag="vq")
                sq1c = bstat.tile([128, 4], F32, tag="sq1")
                sq2c = bstat.tile([128, 4], F32, tag="sq2")
                tmp4 = bstat.tile([128, 4], F32, tag="tmp4")
                ynt = bstat.tile([128, 8], F32, tag="ynt")   # newton y
                yi32 = ynt[:].bitcast(I32)
                aux = bstat.tile([128, 8], F32, tag="aux")
                aux2 = bstat.tile([128, 8], F32, tag="aux2")

                for s in range(NSUB):
                    n0 = j0 + s * SUB
                    xt = xin.tile([128, D_IN], F32, tag="x")
                    nc.sync.dma_start(out=xt[:], in_=x_d[n0:n0 + SUB, :])
                    x_s.append(xt)

                    # --- stats: l0 on DVE bn_stats, l12 fused square-reduce ---
                    st6 = statp.tile([128, 6], F32, tag="st6")
                    mv = statp.tile([128, 2], F32, tag="mv")
                    if not P_SKIP_STATS:
                        nc.vector.bn_stats(out=st6[:], in_=xt[:, 0:M0])
                        nc.vector.bn_aggr(out=mv[:], in_=st6[:])
                    mv_s.append(mv)
                    if not P_SKIP_STATS:
                        nc.gpsimd.tensor_tensor(
                            out=vq[:, s:s + 1], in0=mv[:, 1:2],
                            in1=keps[:, 0:1], op=OP.add)
                        nc.vector.scalar_tensor_tensor(
                            out=dump[:, 0:192], in0=xt[:, 128:320], scalar=1.0,
                            in1=xt[:, 128:320], op0=OP.mult, op1=OP.mult,
                            accum_out=sq1c[:, s:s + 1])
                        nc.vector.scalar_tensor_tensor(
                            out=dump[:, 192:352], in0=xt[:, 320:480], scalar=1.0,
                            in1=xt[:, 320:480], op0=OP.mult, op1=OP.mult,
                            accum_out=sq2c[:, s:s + 1])

                # --- block combine (Pool TT chains) ---
                if P_SKIP_TINY:
                    nc.vector.memset(ynt[:], 1.0)
                else:
                    # q(s): Pool TT chain with const tiles
                    nc.gpsimd.tensor_tensor(out=tmp4[:], in0=sq2c[:], in1=k320[:], op=OP.mult)
                    nc.gpsimd.tensor_tensor(out=sq1c[:], in0=sq1c[:], in1=k384[:], op=OP.mult)
                    nc.gpsimd.tensor_tensor(out=tmp4[:], in0=tmp4[:], in1=sq1c[:], op=OP.add)
                    nc.gpsimd.tensor_tensor(out=vq[:, 4:8], in0=tmp4[:], in1=keps[:, 0:4], op=OP.add)

                    # --- rsqrt(vq): quake seed (DVE int ops) + Newton on Pool ---
                    vi32 = vq[:].bitcast(I32)
                    nc.vector.tensor_scalar(
                        out=yi32, in0=vi32, scalar1=1, scalar2=None,
                        op0=OP.arith_shift_right)
                    nc.vector.scalar_tensor_tensor(
                        out=yi32, in0=magic[:], scalar=0, in1=yi32,
                        op0=OP.bypass, op1=OP.subtract)
                    for _ in range(3):
                        nc.gpsimd.tensor_tensor(out=aux[:], in0=ynt[:], in1=ynt[:], op=OP.mult)
                        nc.gpsimd.tensor_tensor(out=aux2[:], in0=aux[:], in1=vq[:], op=OP.mult)
                        nc.gpsimd.tensor_tensor(out=aux2[:], in0=aux2[:], in1=cneg[:], op=OP.mult)
                        nc.gpsimd.tensor_tensor(out=aux[:], in0=aux2[:], in1=c15[:], op=OP.add)
                        nc.gpsimd.tensor_tensor(out=ynt[:], in0=ynt[:], in1=aux[:], op=OP.mult)
                # ynt cols 0:4 = rstd(s), 4:8 = inv(s)

                # --- normalize node-major (scale fused into cast) ---
                for s in range(NSUB):
                    xc = xbp.tile([128, 128], BF16, tag="xc")
                    nc.vector.tensor_scalar(
                        out=xc[:], in0=x_s[s][:, 0:M0], scalar1=mv_s[s][:, 0:1],
                        scalar2=ynt[:, s:s + 1], op0=OP.subtract, op1=OP.mult)
                    xc_s.append(xc)
                    xb = xbp.tile([128, 352], BF16, tag="xb")
                    nc.gpsimd.tensor_scalar(
                        out=xb[:], in0=x_s[s][:, 128:480],
                        scalar1=ynt[:, 4 + s:5 + s], scalar2=None, op0=OP.mult)
                    xb_s.append(xb)

                # --- transposes to feature-major (PE) ---
                y0p = tpp.tile([128, BLK], BF16, tag="tp")
                t1p = tpp.tile([128, BLK], BF16, tag="tp")
                t2p = tpp.tile([128, BLK], BF16, tag="tp")
                t3p = tpp.tile([128, BLK], BF16, tag="tp")
                for s in range(NSUB):
                    sc = slice(s * SUB, (s + 1) * SUB)
                    P_SKIP_PE or nc.tensor.transpose(out=y0p[:, sc], in_=xc_s[s][:], identity=eye[:])
                    l1 = xb_s[s][:, 0:192].rearrange("p (u m) -> p m u", m=3)
                    l2 = xb_s[s][:, 192:352].rearrange("p (u m) -> p m u", m=5)
                    P_SKIP_PE or nc.tensor.transpose(out=t1p[0:64, sc], in_=l1[:, 0, :], identity=eye[:],
                                        tile_position=(0, 0))
                    P_SKIP_PE or nc.tensor.transpose(out=t1p[64:128, sc], in_=l1[:, 1, :], identity=eye[:],
                                        tile_position=(0, 64))
                    P_SKIP_PE or nc.tensor.transpose(out=t2p[0:64, sc], in_=l1[:, 2, :], identity=eye[:],
                                        tile_position=(0, 0))
                    P_SKIP_PE or nc.tensor.transpose(out=t2p[64:96, sc], in_=l2[:, 0, :], identity=eye[:],
                                        tile_position=(0, 64))
                    P_SKIP_PE or nc.tensor.transpose(out=t2p[96:128, sc], in_=l2[:, 1, :], identity=eye[:],
                                        tile_position=(0, 96))
                    P_SKIP_PE or nc.tensor.transpose(out=t3p[0:32, sc], in_=l2[:, 2, :], identity=eye[:],
                                        tile_position=(0, 0))
                    P_SKIP_PE or nc.tensor.transpose(out=t3p[32:64, sc], in_=l2[:, 3, :], identity=eye[:],
                                        tile_position=(0, 32))
                    P_SKIP_PE or nc.tensor.transpose(out=t3p[64:96, sc], in_=l2[:, 4, :], identity=eye[:],
                                        tile_position=(0, 64))

                # drain yT psum -> sbuf (bf16)
                y0t = ysb.tile([128, BLK], BF16, tag="y0")
                t1t = ysb.tile([128, BLK], BF16, tag="t1")
                t2t = ysb.tile([128, BLK], BF16, tag="t2")
                t3t = ysb.tile([96, BLK], BF16, tag="t3")
                nc.scalar.copy(out=y0t[:], in_=y0p[:])
                nc.scalar.copy(out=t1t[:], in_=t1p[:])
                nc.vector.tensor_copy(out=t2t[:], in_=t2p[:])
                nc.scalar.copy(out=t3t[:], in_=t3p[0:96, :])

                rhs1 = [t1t[0:64, :], t1t[64:128, :], t2t[0:64, :]]
                rhs2 = [t2t[64:96, :], t2t[96:128, :], t3t[0:32, :],
                        t3t[32:64, :], t3t[64:96, :]]

                # --- lin1 l0 + activations ---
                s_sb = []
                tg_sb = []
                for c in range(7):
                    h0p = hpp.tile([128, BLK], F32, tag="h")
                    P_SKIP_PE or nc.tensor.matmul(h0p[:], w0s[:, c, :], y0t[:], start=True, stop=True)
                    if c < 4:
                        st = actp.tile([128, BLK], BF16, tag="s")
                        P_SKIP_ACTF or nc.scalar.activation(out=st[:], in_=h0p[:], func=AF.Silu,
                                             bias=b0s[:, c:c + 1], scale=1.0)
                        s_sb.append(st)
                    else:
                        tg = actp.tile([128, BLK], F32, tag="tg")
                        P_SKIP_ACTF or nc.scalar.activation(out=tg[:], in_=h0p[:], func=AF.Tanh,
                                             bias=b0s[:, c:c + 1], scale=0.5)
                        tg_sb.append(tg)

                # --- lin1 l1/l2 + gating ---
                z1_sb = [[None] * 3 for _ in range(2)]
                for c in range(2):
                    for m in range(3):
                        h1p = hpp.tile([128, BLK], F32, tag="h")
                        base = 0 if m != 1 else 64
                        P_SKIP_PE or nc.tensor.matmul(h1p[:], w1s[base:base + 64, c, :], rhs1[m],
                                         start=True, stop=True, tile_position=(base, 0))
                        zt = zp.tile([128, BLK], BF16, tag="z")
                        P_SKIP_GATING or nc.vector.scalar_tensor_tensor(
                            out=zt[:], in0=tg_sb[c][:], scalar=1.0, in1=h1p[:],
                            op0=OP.add, op1=OP.mult)
                        z1_sb[c][m] = zt
                z2_sb = []
                for m in range(5):
                    h2p = hpp.tile([128, BLK], F32, tag="h")
                    base = [64, 96, 0, 32, 64][m]
                    P_SKIP_PE or nc.tensor.matmul(h2p[:], w2s[base:base + 32, :], rhs2[m],
                                     start=True, stop=True, tile_position=(base, 0))
                    zt = zp.tile([128, BLK], BF16, tag="z")
                    P_SKIP_GATING or nc.vector.scalar_tensor_tensor(
                        out=zt[:], in0=tg_sb[2][:], scalar=1.0, in1=h2p[:],
                        op0=OP.add, op1=OP.mult)
                    z2_sb.append(zt)

                # --- lin2 (feature-major out, m-major rows) ---
                o0p = hpp.tile([128, BLK], F32, tag="h")
                for k in range(4):
                    P_SKIP_PE or nc.tensor.matmul(o0p[:], v0s[:, k, :], s_sb[k][:],
                                     start=(k == 0), stop=(k == 3))
                oap = hpp.tile([128, BLK], F32, tag="h")
                for m in range(2):
                    for k in range(2):
                        P_SKIP_PE or nc.tensor.matmul(oap[m * 64:(m + 1) * 64, :], v1s[:, k, :],
                                         z1_sb[k][m][:], start=(k == 0), stop=(k == 1),
                                         tile_position=(0, m * 64))
                obp = hpp.tile([128, BLK], F32, tag="h")
                for k in range(2):
                    P_SKIP_PE or nc.tensor.matmul(obp[0:64, :], v1s[:, k, :], z1_sb[k][2][:],
                                     start=(k == 0), stop=(k == 1), tile_position=(0, 0))
                P_SKIP_PE or nc.tensor.matmul(obp[64:96, :], v2s[:], z2_sb[0][:], start=True,
                                 stop=True, tile_position=(0, 64))
                P_SKIP_PE or nc.tensor.matmul(obp[96:128, :], v2s[:], z2_sb[1][:], start=True,
                                 stop=True, tile_position=(0, 96))
                ocp = hpp.tile([96, BLK], F32, tag="h")
                for m in range(3):
                    P_SKIP_PE or nc.tensor.matmul(ocp[m * 32:(m + 1) * 32, :], v2s[:],
                                     z2_sb[2 + m][:], start=True, stop=True,
                                     tile_position=(0, m * 32))

                # --- drain + store (device emits t*dx feature-major) ---
                of0 = ofmp.tile([128, BLK], F32, tag="of0")
                ofa = ofmp.tile([128, BLK], F32, tag="ofa")
                ofb = ofmp.tile([128, BLK], F32, tag="ofb")
                ofc = ofmp.tile([96, BLK], F32, tag="ofc")
                nc.scalar.activation(out=of0[:], in_=o0p[:], func=AF.Identity,
                                     bias=c0s[:, 0:1], scale=1.0)
                nc.scalar.copy(out=ofa[:], in_=oap[:])
                nc.scalar.copy(out=ofb[:], in_=obp[:])
                nc.vector.tensor_copy(out=ofc[:], in_=ocp[0:96, :])
                nc.sync.dma_start(out=o_d[0:128, j0:j0 + BLK], in_=of0[:])
                nc.sync.dma_start(out=o_d[128:256, j0:j0 + BLK], in_=ofa[:])
                nc.sync.dma_start(out=o_d[256:384, j0:j0 + BLK], in_=ofb[:])
                nc.sync.dma_start(out=o_d[384:480, j0:j0 + BLK], in_=ofc[:])

    nc.finalize()
    return nc


def _host_weights(inputs):
    bf = ml_dtypes.bfloat16
    t = float(np.tanh(np.float32(inputs["alpha"])))
    nw0 = np.asarray(inputs["nw0"], np.float32)
    nb0 = np.asarray(inputs["nb0"], np.float32)
    nw1 = np.asarray(inputs["nw1"], np.float32)
    nw2 = np.asarray(inputs["nw2"], np.float32)
    W0 = np.asarray(inputs["W0"], np.float32)
    W1 = np.asarray(inputs["W1"], np.float32)
    W2 = np.asarray(inputs["W2"], np.float32)
    V0 = np.asarray(inputs["V0"], np.float32)
    V1 = np.asarray(inputs["V1"], np.float32)
    V2 = np.asarray(inputs["V2"], np.float32)
    b0 = np.asarray(inputs["b0"], np.float32)
    c0 = np.asarray(inputs["c0"], np.float32)

    W0eff = (nw0[:, None] * W0) / S0                      # [128, 896]
    b0eff = b0 + (nb0 @ W0) / S0                          # [896]
    b0act = b0eff.copy()
    b0act[H0:] *= 0.5
    W1eff = (nw1[:, None] * W1) / S1                      # [64, 256]
    W2eff = (nw2[:, None] * W2) / S2                      # [32, 128]
    V0eff = t * V0 / T0                                   # [512, 128]
    V1eff = 0.5 * t * V1 / T1                             # [256, 64]
    V2eff = 0.5 * t * V2 / T2                             # [128, 32]
    c0eff = t * c0                                        # [128]

    w0 = np.ascontiguousarray(W0eff.reshape(128, 7, 128), dtype=bf)
    # w1: chunks along M (256 -> 2x128), replicated x2 along partitions
    w1c = np.stack([W1eff[:, 0:128], W1eff[:, 128:256]], axis=1)  # [64, 2, 128]
    w1 = np.ascontiguousarray(np.concatenate([w1c, w1c], axis=0), dtype=bf)
    w2 = np.ascontiguousarray(np.concatenate([W2eff] * 4, axis=0), dtype=bf)  # [128,128]
    v0 = np.ascontiguousarray(
        V0eff.reshape(4, 128, 128).transpose(1, 0, 2), dtype=bf)  # [128,4,128]
    v1 = np.ascontiguousarray(V1eff.reshape(2, 128, 64).transpose(1, 0, 2), dtype=bf)
    v2 = np.ascontiguousarray(V2eff, dtype=bf)
    b0t = np.ascontiguousarray(b0act.reshape(7, 128).T, dtype=np.float32)  # [128,7]
    c0t = np.ascontiguousarray(c0eff.reshape(128, 1), dtype=np.float32)
    eye = np.ascontiguousarray(np.eye(128), dtype=bf)
    return dict(w0=w0, w1=w1, w2=w2, v0=v0, v1=v1, v2=v2, b0=b0t, c0=c0t, eye=eye)


def kernel(**inputs):
    global _BUILT
    if _BUILT is None:
        _BUILT = _build_bass()
    nc = _BUILT

    x = np.ascontiguousarray(np.asarray(inputs["x"], np.float32))
    wd = _host_weights(inputs)
    in_maps = []
    for c in range(N_CORES):
        m = {"x": np.ascontiguousarray(x[c * NC:(c + 1) * NC, :])}
        m.update(wd)
        in_maps.append(m)

    global LAST_RESULTS
    res = run_bass_kernel_spmd(nc, in_maps, core_ids=list(range(N_CORES)),
                               trace=TRACE, **TRACE_KW)
    LAST_RESULTS = res

    out = np.empty((N_NODES, D_IN), np.float32)
    for c in range(N_CORES):
        o_c = res.results[c]["o"]                      # [480, 8192] = t*dx
        oc = np.empty((NC, D_IN), np.float32)
        oc[:, PERM] = o_c.T
        out[c * NC:(c + 1) * NC, :] = oc
    out += x
    return out


if __name__ == "__main__":
    ins = {k: np.asarray(v) for k, v in np.load(sys.argv[1], allow_pickle=True).item().items()}
    kernel(**ins)

